# revision 51
# baseline (speedup 1.0000x reference)
"""Trainium2 Bass kernel: GQA attention (B=2, S=2048, D=2048, 32 q-heads,
8 kv-heads, head_dim 64, RoPE interleaved, causal) on 8 NeuronCores.

Sharding: tensor-parallel over heads. Core c owns q-heads 4c..4c+3 (= kv head
c) for BOTH batch elements. The kernel streams x one (batch, half) chunk at a
time, projecting q/k/v for that chunk, and INTERLEAVES causal-attention block
work for already-projected query groups between the projection passes so the
PE never idles.

Attention pipeline (per 128-key x 512-query block): scores matmuls write a
PSUM strip, exp is issued IMMEDIATELY on ScalarE (with mask fixups on DVE),
and the attn@V matmuls for a block run TWO blocks later (depth-2 software
pipeline). The 2-deep lookahead gives ScalarE a full block of slack so the
PE never waits on exp; PSUM cost is unchanged (2 score strips in flight).

Output projection: attention outputs ship through an 8-core AllToAll
mid-kernel (token halves u<2), part-A output tiles interleave with the tail
attention groups; the second-half outputs go through two half-size AllToAlls
and part B accumulates the even contraction tiles (already landed) while the
last AllToAll is still in flight, using all 8 PSUM banks (attention pools are
closed by then).

All matmul inputs are bf16 (PSUM accumulation stays fp32). exp runs on
ScalarE with the 1/sqrt(hd) scale folded into the activation's affine
pre-scale. Softmax denominators come from a ones-column accumulated alongside
attn@V; normalization broadcasts the raw sum with a K=1 matmul FIRST, then
reciprocals the [64,512] broadcast.
"""

import numpy as np

B, S, D = 2, 2048, 2048
NH, NKV, HD = 32, 8, 64
THETA = 10000.0
NCORES = 8
NEG = -1.0e30

_BUILT = None


def _swap_mask():
    m = []
    for i in range(16):
        m += [2 * i + 1, 2 * i]
    return m


def _build():
    """Build + compile the SPMD Bass program (once per process)."""
    global _BUILT
    if _BUILT is not None:
        return _BUILT

    from contextlib import ExitStack

    import concourse.tile as tile
    from concourse import bacc, mybir
    from concourse.masks import make_identity

    f32 = mybir.dt.float32
    bf = mybir.dt.bfloat16
    AF = mybir.ActivationFunctionType

    nc = bacc.Bacc(
        "TRN2", target_bir_lowering=False, debug=False, num_devices=NCORES
    )

    xT = nc.dram_tensor("xT", [B, 16, 2, 128, 1024], bf, kind="ExternalInput").ap()
    wqTc = nc.dram_tensor("wqTc", [D, 256], bf, kind="ExternalInput").ap()
    wkvTc = nc.dram_tensor("wkvTc", [D, 128], bf, kind="ExternalInput").ap()
    woT = nc.dram_tensor("woT", [D, D], bf, kind="ExternalInput").ap()
    cosd = nc.dram_tensor("cosd", [128, S], bf, kind="ExternalInput").ap()
    sind = nc.dram_tensor("sind", [128, S], bf, kind="ExternalInput").ap()
    maskd = nc.dram_tensor("maskd", [4, 128, 512], bf, kind="ExternalInput").ap()
    onesd = nc.dram_tensor("onesd", [128, 64], bf, kind="ExternalInput").ap()
    outT = nc.dram_tensor("outT", [D, 512], f32, kind="ExternalOutput").ap()

    SW = _swap_mask()
    SCALE = 1.0 / float(np.sqrt(HD))

    with tile.TileContext(nc) as tc, ExitStack() as top:
        top.enter_context(
            nc.allow_low_precision(reason="bf16 matmul inputs by design")
        )
        res = top.enter_context(tc.tile_pool(name="resident", bufs=1))
        qt = [res.tile([128, B * S], bf, tag=f"qt{p}", name=f"qt{p}") for p in range(2)]
        kt = res.tile([128, B * S], bf, tag="kt")  # kv head, duplicated rows
        vt = [res.tile([128, HD + 1], bf, tag=f"vt{i}", name=f"vt{i}") for i in range(2 * 16)]
        msk = [res.tile([128, 512], bf, tag=f"m{j}", name=f"m{j}") for j in range(4)]
        ones_t = res.tile([128, 64], bf, tag="ones")
        cos_t = res.tile([128, S], bf, tag="cos")
        sin_t = res.tile([128, S], bf, tag="sin")
        ident = res.tile([128, 128], bf, tag="ident")
        wq_t = [res.tile([128, 256], bf, tag=f"wq{d}", name=f"wq{d}") for d in range(16)]
        wkv_t = [res.tile([128, 128], bf, tag=f"wkv{d}", name=f"wkv{d}") for d in range(16)]
        wo_t = [res.tile([128, D], bf, tag=f"wo{e}", name=f"wo{e}") for e in range(16)]
        # attention-output tiles received via AllToAll; one wide tile per
        # token-half so each collective unload is a handful of descriptors
        rh_b = [res.tile([128, 16 * 256], bf, tag=f"rh{h}", name=f"rh{h}") for h in range(2)]
        rh_t = [
            [rh_b[h][:, 256 * e:256 * (e + 1)] for e in range(16)]
            for h in range(2)
        ]

        # weight + first-chunk loads first so the first matmul starts early
        nc.sync.dma_start(out=wkv_t[0][:], in_=wkvTc[0:128, :])
        nc.sync.dma_start(out=wq_t[0][:], in_=wqTc[0:128, :])
        nc.scalar.dma_start(out=ones_t[:], in_=onesd[:])
        make_identity(nc, ident[:])
        # v-tile softmax-denominator ones column: written once, tiles are
        # resident (saves 40 tiny sync-queue DMA descriptors mid-stream)
        for i in range(2 * 16):
            nc.gpsimd.memset(vt[i][:, HD:HD + 1], 1.0)


        dram = top.enter_context(tc.tile_pool(name="dram", bufs=1, space="DRAM"))
        a2a_in = [dram.tile([8, 256, 256], bf, tag=f"a2ain{h}", name=f"a2ain{h}") for h in range(2)]
        a2a_out = [dram.tile([8, 256, 256], bf, tag=f"a2aout{h}", name=f"a2aout{h}") for h in range(2)]
        a2aB_in = [dram.tile([8, 128, 256], bf, tag=f"aBin{p}", name=f"aBin{p}") for p in range(2)]
        a2aB_out = [dram.tile([8, 128, 256], bf, tag=f"aBout{p}", name=f"aBout{p}") for p in range(2)]

        xp = top.enter_context(tc.tile_pool(name="xchunk", bufs=16))
        vstage = top.enter_context(tc.tile_pool(name="vstage", bufs=2))
        rtmp = top.enter_context(tc.tile_pool(name="ropetmp", bufs=2))
        esp = top.enter_context(tc.tile_pool(name="expsbuf", bufs=3))
        nrm = top.enter_context(tc.tile_pool(name="normtmp", bufs=2))
        wos = top.enter_context(tc.tile_pool(name="wosbuf", bufs=6))
        # PSUM (mid section): 'sp' 2x[128,1024]f32 = 4 banks, 'av'
        # 2x[65,512]f32 = 2 banks, 'sm' 2x 2KB slots = 2 banks. These pools
        # close before the tail so part B can hold all 16 accumulators.
        mid = ExitStack()
        psp = mid.enter_context(tc.tile_pool(name="spsum", bufs=2, space="PSUM"))
        avp = mid.enter_context(tc.tile_pool(name="avpsum", bufs=2, space="PSUM"))
        smp = mid.enter_context(tc.tile_pool(name="smpsum", bufs=2, space="PSUM"))

        qv = [
            qt[p][:].rearrange("p (b u i) -> p b u i", b=2, u=4)
            for p in range(2)
        ]

        # ------------- attention emission (generator, 1 block per tick) ----
        # deep-diagonal blocks (j = kb-4u >= 2): queries < 128*j are invalid
        # for EVERY key in the block, so scores/exp/attn@V all run only on
        # the valid column suffix; the ex prefix is never written or read.
        def qoff(u, kb):
            j = kb - 4 * u
            return 128 * j if j >= 2 else 0

        def scores(b, u, p, kb, tag):
            kcol = 2048 * b + 128 * kb
            c = qoff(u, kb)
            sp = psp.tile([128, 1024], f32, tag="sp", name=f"sp{tag}{kb}")
            for hh in range(2):
                r0 = 64 * hh
                nc.tensor.matmul(
                    sp[:, 512 * hh + c:512 * hh + 512],
                    kt[r0:r0 + 64, kcol:kcol + 128],
                    qv[p][r0:r0 + 64, b, u, c:512],
                    start=True, stop=True,
                )
            return sp

        def exp_only(b, u, p, kb, sp, tag):
            ex = esp.tile([128, 1024], bf, tag="ex", name=f"ex{tag}{kb}")
            c = qoff(u, kb)
            if c:
                spv = sp[:].rearrange("p (h i) -> p h i", h=2)[:, :, c:512]
                exv = ex[:].rearrange("p (h i) -> p h i", h=2)[:, :, c:512]
                nc.scalar.activation(exv, spv, AF.Exp, scale=SCALE)
            else:
                nc.scalar.activation(ex[:], sp[:], AF.Exp, scale=SCALE)
            return ex

        def mask_only(b, u, p, kb, ex, tag):
            j = kb - 4 * u
            if j < 0:
                return
            c = qoff(u, kb)
            if c:
                for hh in range(2):
                    c0 = 512 * hh
                    nc.vector.tensor_mul(
                        ex[:, c0 + c:c0 + c + 128], ex[:, c0 + c:c0 + c + 128],
                        msk[j][:, c:c + 128]
                    )
            else:
                # zero the causally-invalid staircase (cols < 128*(j+1))
                w = 128 * (j + 1)
                for hh in range(2):
                    c0 = 512 * hh
                    nc.vector.tensor_mul(
                        ex[:, c0:c0 + w], ex[:, c0:c0 + w], msk[j][:, 0:w]
                    )

        def av_mm(b, u, p, kb, ex, av, first, last):
            v_ = vt[16 * b + kb]
            c = qoff(u, kb)
            for hh in range(2):
                nc.tensor.matmul(
                    av[hh][:, c:512], v_[:],
                    ex[:, 512 * hh + c:512 * hh + 512],
                    start=first, stop=last,
                )

        def make_norm(b, u, p, av, tag):
            def avcopy():
                cps = []
                for hh in range(2):
                    cp = nrm.tile([65, 512], bf, tag="cp", name=f"cp{tag}{hh}")
                    nc.vector.tensor_copy(cp[:], av[hh][0:65, :])
                    cps.append(cp)
                return (cps, None)

            def recip(cpb):
                cps, _ = cpb
                rrs = []
                for hh in range(2):
                    bcp = smp.tile([128, 512], f32, tag="sm", name=f"bc{tag}{hh}")
                    nc.tensor.matmul(
                        bcp[0:64, :], ones_t[64:65, 0:64], cps[hh][64:65, :],
                        start=True, stop=True,
                    )
                    rr = nrm.tile([64, 512], f32, tag="rr", name=f"rr{tag}{hh}")
                    nc.vector.reciprocal_approx_fast(rr[:], bcp[0:64, :])
                    rrs.append(rr)
                return rrs

            def rest(cps, rrs):
                for hh in range(2):
                    at_ = nrm.tile([64, 512], bf, tag="at", name=f"at{tag}{hh}")
                    nc.vector.tensor_mul(at_[:], cps[hh][0:64, :], rrs[hh][:])
                    # both hf halves in one descriptor (dst stride = 1 in dim 0)
                    dst = 4 * b + 2 * (u % 2)
                    atv = at_[:].rearrange("r (f c) -> r f c", f=2)
                    if u // 2 == 0:
                        r0 = 128 * p + 64 * hh
                        nc.sync.dma_start(
                            out=a2a_in[0][dst:dst + 2, r0:r0 + 64, :]
                            .rearrange("f r c -> r f c"),
                            in_=atv,
                        )
                    else:
                        nc.sync.dma_start(
                            out=a2aB_in[p][dst:dst + 2, 64 * hh:64 * hh + 64, :]
                            .rearrange("f r c -> r f c"),
                            in_=atv,
                        )

            return avcopy, recip, rest

        def emit_a2a(h):
            nc.gpsimd.collective_compute(
                "AllToAll",
                mybir.AluOpType.bypass,
                replica_groups=[list(range(8))],
                ins=[a2a_in[h][:].opt()],
                outs=[a2a_out[h][:].opt()],
            )

        def unload_a2a(h):
            # emitted LATE (when the collective is long done) so these
            # descriptors never head-of-line block the sync queue
            for c4 in range(4):
                nc.sync.dma_start(
                    out=rh_b[h][:, 1024 * c4:1024 * (c4 + 1)]
                    .rearrange("p (s t c) -> p s t c", s=2, t=2),
                    in_=a2a_out[h][2 * c4:2 * c4 + 2]
                    .rearrange("s (t p) c -> p s t c", t=2),
                )

        def emit_a2aB(p):
            nc.gpsimd.collective_compute(
                "AllToAll",
                mybir.AluOpType.bypass,
                replica_groups=[list(range(8))],
                ins=[a2aB_in[p][:].opt()],
                outs=[a2aB_out[p][:].opt()],
            )

        def unload_a2aB(p, queue):
            for s_ in range(8):
                e = 2 * s_ + p
                queue.dma_start(
                    out=rh_b[1][:, 256 * e:256 * (e + 1)], in_=a2aB_out[p][s_]
                )

        def phase3_unit(h, m):
            po = smp.tile([128, 256], f32, tag="sm", name=f"po{h}{m}")
            for i_, e in enumerate(range(16)):
                nc.tensor.matmul(
                    po[:], wo_t[e][:, 128 * m:128 * (m + 1)], rh_t[h][e][:],
                    start=(i_ == 0), stop=(i_ == 15),
                )
            os_ = wos.tile([128, 256], f32, tag="os")
            nc.vector.tensor_copy(os_[:], po[:])
            nc.sync.dma_start(
                out=outT[128 * m:128 * (m + 1), 256 * h:256 * h + 256], in_=os_[:]
            )

        # p-group order by causal availability: (0,u<=1) need only C0;
        # (1,u<=1) need C1; (0,u>=2) need C2; (1,u>=2) need C3.
        # Second half: ALL p=0 groups run before p=1 so the a2aB(0)
        # collective (p=0 payload) fires ~25 ticks before the stream ends
        # and part B's even tiles land while p=1 attention still runs.
        GROUPS = (
            [(b, u, p)
             for (b, u) in [(0, 0), (0, 1), (1, 0), (1, 1)]
             for p in range(2)]
            + [(0, 2, 0), (0, 3, 0), (0, 2, 1), (0, 3, 1),
               (1, 2, 0), (1, 3, 0), (1, 2, 1), (1, 3, 1)]
        )
        BLOCKS = []
        for gi, (b, u, p) in enumerate(GROUPS):
            nkb = 4 * u + 4
            for i in range(nkb):
                BLOCKS.append((gi, b, u, p, i, i == 0, i == nkb - 1))

        PA_START, PA_STEP = 132, 2  # part-A units start after A2A#1 lands

        ticker = {"t": 0}
        todo = []       # (due_tick, fn) scheduled norm-chain fragments
        group_st = {}   # gi -> (av, avcopy, recip, rest)
        pipe = []       # blocks issued (scores+exp) but attn@V not yet run
        p3 = [0]

        def run_todos():
            t = ticker["t"]
            due = sorted([x for x in todo if x[0] <= t], key=lambda x: x[0])
            for item in due:
                todo.remove(item)
                item[1]()

        def do_rest(gi, cpb, rrs):
            _, _, _, rest = group_st[gi]
            rest(cpb[0], rrs)
            if gi == 7:
                emit_a2a(0)
                todo.append((ticker["t"] + 58, lambda: unload_a2a(0)))
            elif gi == 13:
                emit_a2aB(0)

        def do_recip(gi, cpb):
            _, _, recip, _ = group_st[gi]
            rrs = recip(cpb)
            todo.append((ticker["t"] + 2, lambda: do_rest(gi, cpb, rrs)))

        def pop_block():
            gi, b, u, p, kb, first, last, ex = pipe.pop(0)
            av, avcopy, _, _ = group_st[gi]
            av_mm(b, u, p, kb, ex, av, first, last)
            if last:
                # avcopy emitted here so it lands AHEAD of the current
                # block's mask multiplies in the Vector queue (the next
                # group's first attn@V waits on it through the av slot)
                cpb = avcopy()
                todo.append((ticker["t"] + 2, lambda: do_recip(gi, cpb)))

        def attn_stream():
            for (gi, b, u, p, kb, first, last) in BLOCKS:
                tag = f"{b}{u}{p}"
                if first:
                    av = [
                        avp.tile([HD + 1, 512], f32, tag="av", name=f"av{tag}{hh}")
                        for hh in range(2)
                    ]
                    avcopy, recip, rest = make_norm(b, u, p, av, tag)
                    group_st[gi] = (av, avcopy, recip, rest)
                sp = scores(b, u, p, kb, tag)
                ex = exp_only(b, u, p, kb, sp, tag)
                pipe.append((gi, b, u, p, kb, first, last, ex))
                if len(pipe) > 2:
                    pop_block()
                mask_only(b, u, p, kb, ex, tag)
                run_todos()
                t = ticker["t"]
                if t >= PA_START and (t - PA_START) % PA_STEP == 0 and p3[0] < 16:
                    phase3_unit(0, p3[0])
                    p3[0] += 1
                ticker["t"] = t + 1
                yield

        stream = attn_stream()
        ticks_left = [0]

        def tick():
            if ticks_left[0] <= 0:
                return
            try:
                next(stream)
                ticks_left[0] -= 1
            except StopIteration:
                ticks_left[0] = 0

        # ------------- projection chunk (b, half) with interleave ---------
        def emit_x(b, half):
            xt = []
            for d in range(16):
                x_ = xp.tile([128, 1024], bf, tag="x", name=f"x{b}{half}{d}")
                nc.sync.dma_start(out=x_[:], in_=xT[b, d, half])
                xt.append(x_)
            return xt

        def rope_q(b, half, s, p, pq, col):
            ccol = col % S
            t1 = rtmp.tile([128, 512], f32, tag="t1")
            nc.vector.tensor_mul(t1[:], pq[:], cos_t[:, ccol:ccol + 512])
            sw = rtmp.tile([128, 512], f32, tag="sw")
            nc.vector.stream_shuffle(sw[:], pq[:], SW)
            t2 = rtmp.tile([128, 512], f32, tag="t2")
            nc.vector.tensor_mul(t2[:], sw[:], sin_t[:, ccol:ccol + 512])
            nc.vector.tensor_add(qt[p][:, col:col + 512], t1[:], t2[:])

        def rope_k_v(b, half, s, pkv, col):
            ccol = col % S
            # v: copy + PE-transpose 4 key-blocks of 128
            vs = vstage.tile([128, 512], bf, tag="vs")
            nc.scalar.copy(vs[64:128, :], pkv[64:128, :])
            t1 = rtmp.tile([128, 512], f32, tag="t1")
            nc.vector.tensor_mul(t1[0:64, :], pkv[0:64, :], cos_t[0:64, ccol:ccol + 512])
            sw = rtmp.tile([128, 512], f32, tag="sw")
            nc.vector.stream_shuffle(sw[0:64, :], pkv[0:64, :], SW)
            t2 = rtmp.tile([128, 512], f32, tag="t2")
            nc.vector.tensor_mul(t2[0:64, :], sw[0:64, :], sin_t[0:64, ccol:ccol + 512])
            nc.vector.tensor_add(kt[0:64, col:col + 512], t1[0:64, :], t2[0:64, :])
            nc.sync.dma_start(
                out=kt[64:128, col:col + 512], in_=kt[0:64, col:col + 512]
            )
            for j in range(4):
                ptv = smp.tile([128, HD], bf, tag="sm", name=f"tv{b}{half}{s}{j}")
                nc.tensor.transpose(
                    ptv[:], vs[64:128, 128 * j:128 * (j + 1)], ident[64:128, 64:128]
                )
                kb = 8 * half + 4 * s + j
                nc.scalar.copy(vt[16 * b + kb][:, 0:HD], ptv[:])

        def proj_chunk0(xt):
            # C0: no attention to interleave; issue all 6 accumulations per
            # arriving chunk so the PE keeps pace with the cold DMA stream
            b = half = 0
            pkv = [smp.tile([128, 512], f32, tag="sm", name=f"c0kv{s}") for s in range(2)]
            pq0 = psp.tile([128, 1024], f32, tag="sp", name="c0q0")
            pq1 = psp.tile([128, 1024], f32, tag="sp", name="c0q1")
            for d in range(16):
                for s in range(2):
                    xs = xt[d][:, 512 * s:512 * s + 512]
                    nc.tensor.matmul(pkv[s][:], wkv_t[d][:], xs,
                                     start=(d == 0), stop=(d == 15))
                    nc.tensor.matmul(pq0[:, 512 * s:512 * s + 512], wq_t[d][:, 0:128],
                                     xs, start=(d == 0), stop=(d == 15))
                    nc.tensor.matmul(pq1[:, 512 * s:512 * s + 512], wq_t[d][:, 128:256],
                                     xs, start=(d == 0), stop=(d == 15))
            for s in range(2):
                rope_k_v(b, half, s, pkv[s], 512 * s)
            for s in range(2):
                rope_q(b, half, s, 0, pq0[:, 512 * s:512 * s + 512], 512 * s)
                rope_q(b, half, s, 1, pq1[:, 512 * s:512 * s + 512], 512 * s)

        def proj_chunk(b, half, nticks, xt=None):
            ticks_left[0] = nticks
            if xt is None:
                xt = emit_x(b, half)
            for s in range(2):
                col = 2048 * b + 1024 * half + 512 * s

                # kv pass
                pkv = smp.tile([128, 512], f32, tag="sm", name=f"pkv{b}{half}{s}")
                for d in range(16):
                    nc.tensor.matmul(
                        pkv[:], wkv_t[d][:], xt[d][:, 512 * s:512 * s + 512],
                        start=(d == 0), stop=(d == 15),
                    )
                    if d % 2 == 1:
                        tick()
                rope_k_v(b, half, s, pkv, col)
                tick()

                # q passes (head-pair p = 0, 1)
                for p in range(2):
                    pq = smp.tile([128, 512], f32, tag="sm", name=f"pq{b}{half}{s}{p}")
                    for d in range(16):
                        nc.tensor.matmul(
                            pq[:], wq_t[d][:, 128 * p:128 * p + 128],
                            xt[d][:, 512 * s:512 * s + 512],
                            start=(d == 0), stop=(d == 15),
                        )
                        if d % 4 == 3:
                            tick()
                    rope_q(b, half, s, p, pq, col)
                    tick()

        # wo loads stream in the background once the first chunk is queued
        # (scalar queue: keeps the sync queue free for x-chunk streaming)
        def load_wo(lo, hi):
            for e in range(lo, hi):
                nc.scalar.dma_start(out=wo_t[e][:], in_=woT[128 * e:128 * (e + 1), :])

        # chunks in causal-availability order; tick budgets:
        # after C0: G(0,0)=8 blocks(+2 transitions); after C1: G(1,0)+G(0,1);
        # after C2: G(1,1)+G(0,2); then drain
        xt0 = []
        for d in range(16):
            if d >= 1:
                nc.sync.dma_start(out=wkv_t[d][:], in_=wkvTc[128 * d:128 * (d + 1), :])
                nc.sync.dma_start(out=wq_t[d][:], in_=wqTc[128 * d:128 * (d + 1), :])
            x_ = xp.tile([128, 1024], bf, tag="x", name=f"x00{d}")
            if d == 0:
                # first tile split across 4 descriptors/queues: the very
                # first matmul otherwise waits ~11us for one 256KB queue
                for qq in range(4):
                    nc.sync.dma_start(
                        out=x_[:, 256 * qq:256 * (qq + 1)],
                        in_=xT[0, d, 0][:, 256 * qq:256 * (qq + 1)],
                    )
            else:
                nc.sync.dma_start(out=x_[:], in_=xT[0, d, 0])
            xt0.append(x_)
            if d == 7:
                # rope tables early enough for chunk0's rope (~40us)
                nc.sync.dma_start(out=cos_t[:], in_=cosd[:])
                nc.sync.dma_start(out=sin_t[:], in_=sind[:])
        for j in range(4):
            nc.sync.dma_start(out=msk[j][:], in_=maskd[j])
        proj_chunk0(xt0)
        proj_chunk(1, 0, 28)
        load_wo(0, 8)
        proj_chunk(0, 1, 36)
        load_wo(8, 16)
        proj_chunk(1, 1, 24)
        # drain the rest of attention + interleaved phase-3 part A
        ticks_left[0] = 10 ** 9
        for _ in stream:
            pass
        # flush the depth-2 pipe and the deferred norm chain
        while pipe:
            pop_block()
            ticker["t"] += 1
            run_todos()
        while todo:
            ticker["t"] += 1
            run_todos()
        while p3[0] < 16:
            phase3_unit(0, p3[0])
            p3[0] += 1

        # ---------------- tail: second AllToAll + part B -------------------
        # attention-phase PSUM pools close; part B gets all 8 banks and
        # accumulates the even contraction tiles (landed with a2aB(0)) while
        # a2aB(1) is still in flight.
        mid.close()
        emit_a2aB(1)
        unload_a2aB(0, nc.sync)
        tlp = top.enter_context(tc.tile_pool(name="tailpsum", bufs=8, space="PSUM"))
        pot = [tlp.tile([128, 512], f32, tag="pb", name=f"pb{j}") for j in range(8)]

        def pdst(m):
            return pot[m // 2][:, 256 * (m % 2):256 * (m % 2) + 256]

        # PSUM start=True zeroes the whole 2KB bank (ZERO_REGION_SIZE): issue
        # exactly one start per bank (the even-m unit); the odd-m unit's
        # first write lands on still-pending-zero bytes and overwrites.
        for m in range(16):
            for i_, e in enumerate(range(0, 16, 2)):
                nc.tensor.matmul(
                    pdst(m), wo_t[e][:, 128 * m:128 * (m + 1)], rh_t[1][e][:],
                    start=(i_ == 0 and m % 2 == 0), stop=False,
                    skip_group_check=True,
                )
        unload_a2aB(1, nc.sync)
        for m in range(16):
            for i_, e in enumerate(range(1, 16, 2)):
                nc.tensor.matmul(
                    pdst(m), wo_t[e][:, 128 * m:128 * (m + 1)], rh_t[1][e][:],
                    start=False, stop=(i_ == 7),
                    skip_group_check=True,
                )
            os_ = wos.tile([128, 256], f32, tag="os")
            nc.vector.tensor_copy(os_[:], pdst(m))
            nc.sync.dma_start(
                out=outT[128 * m:128 * (m + 1), 256:512], in_=os_[:]
            )

    nc.compile()
    _BUILT = nc
    return nc


def _host_inputs(x, wq, wk, wv, wo):
    """Per-core input maps (host-side layout prep only, no math on x)."""
    import ml_dtypes

    bf16 = ml_dtypes.bfloat16
    x = np.ascontiguousarray(x, dtype=np.float32)
    xT3 = x.transpose(0, 2, 1)
    xT = np.ascontiguousarray(
        xT3.reshape(B, 16, 128, 2, 1024).transpose(0, 1, 3, 2, 4).astype(bf16)
    )
    woT = np.ascontiguousarray(np.asarray(wo, np.float32).T.astype(bf16))

    inv = THETA ** (-np.arange(32, dtype=np.float64) / 32.0)
    ang = np.outer(inv, np.arange(S, dtype=np.float64))  # [32, S]
    cos1 = np.cos(ang).astype(np.float32)
    sin1 = np.sin(ang).astype(np.float32)
    pairs = (np.arange(128) % 64) // 2
    signs = np.where(np.arange(128) % 2 == 0, -1.0, 1.0).astype(np.float32)
    cosd = np.ascontiguousarray(cos1[pairs].astype(bf16))
    sind = np.ascontiguousarray((sin1[pairs] * signs[:, None]).astype(bf16))

    k_i = np.arange(128)[:, None]
    q_i = np.arange(512)[None, :]
    maskd = np.stack(
        [np.where(q_i >= k_i + 128 * j, 1.0, 0.0) for j in range(4)]
    ).astype(bf16)
    onesd = np.ones((128, 64), bf16)

    wq = np.asarray(wq, np.float32)
    wk = np.asarray(wk, np.float32)
    wv = np.asarray(wv, np.float32)
    in_maps = []
    for c in range(NCORES):
        wqTc = np.ascontiguousarray(wq[256 * c:256 * (c + 1), :].T.astype(bf16))
        wkvTc = np.ascontiguousarray(
            np.concatenate(
                [wk[64 * c:64 * (c + 1), :].T, wv[64 * c:64 * (c + 1), :].T], axis=1
            ).astype(bf16)
        )
        in_maps.append(
            {
                "xT": xT, "wqTc": wqTc, "wkvTc": wkvTc, "woT": woT,
                "cosd": cosd, "sind": sind, "maskd": maskd, "onesd": onesd,
            }
        )
    return in_maps


def run(x, wq, wk, wv, wo, trace=False):
    """Build, run on 8 cores, assemble full output. Returns (out, results)."""
    from concourse.bass_utils import run_bass_kernel_spmd

    nc = _build()
    in_maps = _host_inputs(x, wq, wk, wv, wo)
    r = run_bass_kernel_spmd(nc, in_maps, list(range(NCORES)), trace=trace)
    out = np.empty((B, S, D), np.float32)
    for c in range(NCORES):
        b, j = c // 4, c % 4
        oT = r.results[c]["outT"]
        qa = 512 * (j // 2) + 256 * (j % 2)
        qb = 1024 + 512 * (j // 2) + 256 * (j % 2)
        out[b, qa:qa + 256, :] = oT[:, 0:256].T
        out[b, qb:qb + 256, :] = oT[:, 256:512].T
    return out, r


def kernel(x, wq, wk, wv, wo):
    out, _ = run(x, wq, wk, wv, wo, trace=False)
    return out
